# revision 17
# baseline (speedup 1.0000x reference)
"""Self-contained Trainium2 attention-block kernel (8 NeuronCores, SPMD).

Problem: x[4,4096,128], Wq/Wk[64,128], Wv[128,128] ->
  softmax((x Wq^T)(x Wk^T)^T / 8) (x Wv^T)   -> [4,4096,128] f32

Sharding: data-parallel over batch (4) x query-halves (2) = 8 cores.
Each core: q rows 2048, full K (4096) recomputed locally. No collectives.

v2 design (vs v1 which was ScalarE-bound at ~117us):
  - P*V reassociated as (P*X)*Wv^T: the per-chunk PV matmul uses raw x
    chunks as stationary weights (no V projection, no V casts); one
    [d,v] Wv matmul per q-block at the end; output DMA'd f32 from PSUM.
  - softmax denominator D via PE: accumulating ones-matmuls into a psum
    bank (rhs = bf16 pair-sums t1 for the first TJ groups, raw p chunks
    for the rest) instead of a deep DVE add-tree.
  - exp split across engines: every 4th group's exp runs on the DVE as
    a one-instruction Schraudolph bit-trick (u16 = s*A + B viewed as
    bf16 ~= exp(s/8)); the rest stay on ScalarE's table exp.
  - PE warm-up spin during the prologue so HAM reaches K=8/8 before
    steady state, plus weights-first DMA order and just-in-time
    projection emission.
"""

import sys

sys.path.insert(0, "/opt/trn_rl_repo")

from contextlib import ExitStack

import ml_dtypes
import numpy as np

import concourse.bass as bass  # noqa: F401
import concourse.bacc as bacc
import concourse.tile as tile
from concourse import mybir
from concourse.bass_utils import run_bass_kernel_spmd

BF16 = mybir.dt.bfloat16
F32 = mybir.dt.float32
U16 = mybir.dt.uint16
NPBF16 = ml_dtypes.bfloat16

B, S, D, A = 4, 4096, 128, 64
NQ = S // 2          # q rows per core
QB = 512             # q block (psum bank free size)
KC = 128             # k chunk (matmul contraction tile)
NKC = S // KC        # 32 chunks
NQB = NQ // QB       # 4 q blocks
GROUP = 2            # k chunks per exp group ([128,1024] psum tile)
NGRP = NKC // GROUP  # 16 groups per block
EXP = mybir.ActivationFunctionType.Exp

# tuning knobs
DVE_EXP_GROUPS = (1, 5, 9, 13)  # groups per qblock: exp on DVE (Schraudolph)
DMM_LAG = 4       # groups between a t2 tile's last input and its D-matmul
FINISH_DELAY = 6  # groups into the next block before finishing a block
N_WARM = 14       # prologue PE warm-up matmuls (N=512)

# Schraudolph constants: u16 = round(s * SCH_A + SCH_B) viewed as bf16
# approximates exp(s/8).  t = s*log2(e)/8; bits = 128*t + (127*128 - C).
SCH_A = 128 * np.log2(np.e) / 8          # 23.083120654223414
SCH_B = 16256.0 - 7.5                    # C=7.5 splits round/trunc modes

_CACHED_NC = None


def _log(msg):
    import time as _t
    print(f"[kernel {_t.strftime('%H:%M:%S')}] {msg}", file=sys.stderr, flush=True)


def build_nc():
    _log("build_nc: tracing graph")
    nc = bacc.Bacc(
        "TRN2", target_bir_lowering=False, debug=False,
        enable_asserts=False, num_devices=8,
    )
    xT = nc.dram_tensor("xT", [D, S], BF16, kind="ExternalInput").ap()
    xc = nc.dram_tensor("xc", [128, S], BF16, kind="ExternalInput").ap()
    xqT = nc.dram_tensor("xqT", [D, NQ], BF16, kind="ExternalInput").ap()
    wqTd = nc.dram_tensor("wqTd", [D, 128], BF16, kind="ExternalInput").ap()
    wkTd = nc.dram_tensor("wkTd", [D, 128], BF16, kind="ExternalInput").ap()
    wvT = nc.dram_tensor("wvT", [D, D], BF16, kind="ExternalInput").ap()
    # outT layout [v, q] f32; host transposes during gather
    out = nc.dram_tensor("out", [D, NQ], F32, kind="ExternalOutput").ap()

    with tile.TileContext(nc) as tc, ExitStack() as ctx:
        persist = ctx.enter_context(tc.tile_pool(name="persist", bufs=1))
        # PSUM: st 2x(2 banks) + px 2x(1 bank) + misc 4x(1 bank shared pairwise)
        ps_st = ctx.enter_context(tc.tile_pool(name="ps_st", bufs=2, space="PSUM"))
        ps_px = ctx.enter_context(tc.tile_pool(name="ps_px", bufs=2, space="PSUM"))
        ps_ms = ctx.enter_context(tc.tile_pool(name="ps_ms", bufs=1, space="PSUM"))
        ppool = ctx.enter_context(tc.tile_pool(name="ppool", bufs=12))
        tpool = ctx.enter_context(tc.tile_pool(name="tpool", bufs=10))
        mpool = ctx.enter_context(tc.tile_pool(name="mpool", bufs=4))

        # ---- persistent SBUF ----
        wq_s = persist.tile([D, 128], BF16, tag="wq_s")
        wk_s = persist.tile([D, 128], BF16, tag="wk_s")
        wv_s = persist.tile([D, D], BF16, tag="wv_s")
        ones_s = persist.tile([128, 128], BF16, tag="ones_s")
        xqT_s = persist.tile([D, NQ], BF16, tag="xqT_s")
        xT_s = persist.tile([D, S], BF16, tag="xT_s")
        xc_s = persist.tile([128, S], BF16, tag="xc_s")
        KT_s = persist.tile([128, S], BF16, tag="KT_s")   # duplicated halves
        QT_s = persist.tile([128, NQ], BF16, tag="QT_s")  # duplicated halves

        # ones for the D-matmuls needs no DMA
        nc.gpsimd.memset(ones_s[:], 1.0)

        # input DMAs spread across four queues so they run in parallel;
        # the chunks the first projections need go first on each queue
        nc.sync.dma_start(wq_s[:], wqTd[:])
        nc.sync.dma_start(wk_s[:], wkTd[:])
        nc.sync.dma_start(xqT_s[:, 0:QB], xqT[:, 0:QB])
        for j in range(4):
            nc.sync.dma_start(xT_s[:, j * 1024:(j + 1) * 1024],
                              xT[:, j * 1024:(j + 1) * 1024])
            nc.sync.dma_start(xc_s[:, j * 1024:(j + 1) * 1024],
                              xc[:, j * 1024:(j + 1) * 1024])
        nc.sync.dma_start(xqT_s[:, QB:], xqT[:, QB:])
        nc.sync.dma_start(wv_s[:], wvT[:])

        # prewarm the exp table (ScalarE) off the critical path
        warm = persist.tile([1, 1], F32, tag="warm")
        nc.gpsimd.memset(warm[:], 1.0)
        warm2 = persist.tile([1, 1], F32, tag="warm2")
        nc.scalar.activation(warm2[:], warm[:], EXP)


        # ---- projections (just-in-time emission below for later chunks) ----
        def proj_mm(dst, w, src_slice, cp=None, pool=None):
            if pool is None:
                pt = ps_ms.tile([128, QB], F32, tag="pj", bufs=1)
            else:
                pt = pool.tile([128, QB], F32, tag="st")
            nc.tensor.matmul(pt[:], w, src_slice, start=True, stop=True)
            (cp or nc.scalar.copy)(dst, pt[:])

        # prologue projections rotate through the 2-slot st pool so the
        # matmul->cast->matmul chain pipelines instead of serializing on
        # the single pj slot
        proj_mm(QT_s[:, 0:QB], wq_s[:], xqT_s[:, 0:QB],
                cp=nc.vector.tensor_copy, pool=ps_st)
        proj_mm(KT_s[:, 0:QB], wk_s[:], xT_s[:, 0:QB],
                cp=nc.vector.tensor_copy, pool=ps_st)
        proj_mm(KT_s[:, QB:2 * QB], wk_s[:], xT_s[:, QB:2 * QB],
                cp=nc.vector.tensor_copy, pool=ps_st)
        kt_done = 2
        qt_done = 1

        # ---- attention: flat software pipeline over (qblock, group) ----
        ALL = [(qb, g) for qb in range(NQB) for g in range(NGRP)]

        def emit_st(qb, g):
            q0 = qb * QB
            st = ps_st.tile([128, GROUP * QB], F32, tag="st")
            for i in range(GROUP):
                kc = g * GROUP + i
                h = kc % 2  # row-tile half: concurrent 64-contraction pairs
                lhsT = KT_s[h * 64:(h + 1) * 64, kc * KC:(kc + 1) * KC]
                rhs = QT_s[h * 64:(h + 1) * 64, q0:q0 + QB]
                nc.tensor.matmul(st[:, i * QB:(i + 1) * QB], lhsT, rhs,
                                 start=True, stop=True)
            return st

        st_tiles = {}
        st_tiles[ALL[0]] = emit_st(*ALL[0])
        st_tiles[ALL[1]] = emit_st(*ALL[1])

        px_tiles = {}    # per-qblock PX^T [d, q] psum accumulators
        dps_tiles = {}   # per-qblock D psum accumulators (partition-broadcast)
        t1_tiles = {}    # (qb, g) -> bf16 chunk-pair sums (DVE)
        pending = {}     # emission idx -> list of closures (lagged work)

        def emit_dmm(qb, j):
            """t2 pair-sum (GpSimd) + accumulating ones-matmul (PE)."""
            if qb not in dps_tiles:
                dps_tiles[qb] = ps_ms.tile([128, QB], F32, tag="dps",
                                           name=f"dps{qb}", bufs=1)
            t2 = tpool.tile([128, QB], BF16, tag="t2", bufs=6,
                            name=f"t2_{qb}_{j}")
            nc.gpsimd.tensor_add(t2[:], t1_tiles.pop((qb, 2 * j))[:],
                                 t1_tiles.pop((qb, 2 * j + 1))[:])
            dps = dps_tiles[qb]
            nc.tensor.matmul(dps[:], ones_s[:], t2[:],
                             start=(j == 0), stop=(j == NGRP // 2 - 1))

        def finish_block(qb):
            q0 = qb * QB
            dinvb = mpool.tile([128, QB], F32, tag="dinvb")
            nc.vector.reciprocal_approx_fast(dinvb[:], dps_tiles.pop(qb)[:])
            pxn = mpool.tile([128, QB], BF16, tag="pxn")
            nc.vector.tensor_mul(pxn[:], px_tiles.pop(qb)[:], dinvb[:])
            po = ps_ms.tile([128, QB], F32, tag="pj", name=f"po{qb}", bufs=1)
            nc.tensor.matmul(po[:], wv_s[:], pxn[:], start=True, stop=True)
            ot = mpool.tile([128, QB], F32, tag="ot")
            nc.scalar.copy(ot[:], po[:])
            nc.sync.dma_start(out[:, q0:q0 + QB], ot[:])

        for idx, (qb, g) in enumerate(ALL):
            # lagged t2-adds (GpSimd) + D-matmuls (PE) scheduled for this slot
            for fn in pending.pop(idx, ()):
                fn()

            st = st_tiles.pop((qb, g))
            p = ppool.tile([128, GROUP * QB], BF16, tag="p")
            if g in DVE_EXP_GROUPS:
                # Schraudolph exp on DVE: p_bits = s*A + B, u16-converted
                nc.vector.tensor_scalar(
                    p[:].bitcast(U16), st[:], SCH_A, SCH_B,
                    mybir.AluOpType.mult, mybir.AluOpType.add)
            else:
                nc.scalar.activation(p[:], st[:], EXP, scale=0.125)

            if idx + 2 < len(ALL):
                st_tiles[ALL[idx + 2]] = emit_st(*ALL[idx + 2])

            if qb not in px_tiles:
                px_tiles[qb] = ps_px.tile([128, QB], F32, tag="px",
                                          name=f"px{qb}")
            px = px_tiles[qb]
            for i in range(GROUP):
                kc = g * GROUP + i
                nc.tensor.matmul(px[:], xc_s[:, kc * KC:(kc + 1) * KC],
                                 p[:, i * QB:(i + 1) * QB],
                                 start=(kc == 0), stop=(kc == NKC - 1))

            # level-1 chunk-pair sum on DVE; level-2 + D-matmul lag behind
            t1 = tpool.tile([128, QB], BF16, tag="t1")
            nc.vector.tensor_add(t1[:], p[:, 0:QB], p[:, QB:2 * QB])
            t1_tiles[(qb, g)] = t1
            if g % 2 == 1:
                j = g // 2
                pending.setdefault(idx + DMM_LAG, []).append(
                    lambda qb=qb, j=j: emit_dmm(qb, j))

            # just-in-time projections: KT chunk j feeds ST groups 2j..2j+1
            # (emitted 2 ahead), QT block j feeds q-block j
            need_kt = min(8, (idx + 3) // 2 + 1)
            while kt_done < need_kt:
                proj_mm(KT_s[:, kt_done * QB:(kt_done + 1) * QB], wk_s[:],
                        xT_s[:, kt_done * QB:(kt_done + 1) * QB])
                kt_done += 1
            need_qt = min(NQB, (idx + 3) // NGRP + 1)
            while qt_done < need_qt:
                proj_mm(QT_s[:, qt_done * QB:(qt_done + 1) * QB], wq_s[:],
                        xqT_s[:, qt_done * QB:(qt_done + 1) * QB])
                qt_done += 1

            if g == FINISH_DELAY - 1 and qb > 0:
                finish_block(qb - 1)

        for idx in sorted(k for k in pending if k >= len(ALL)):
            for fn in pending.pop(idx):
                fn()
        finish_block(NQB - 1)

    _log("build_nc: bacc compile")
    nc.compile()
    _log("build_nc: done")
    return nc


def _host_prep(x, Wq, Wk, Wv):
    x = np.asarray(x, dtype=np.float32)
    Wq = np.asarray(Wq, dtype=np.float32)
    Wk = np.asarray(Wk, dtype=np.float32)
    Wv = np.asarray(Wv, dtype=np.float32)
    wqTd = np.ascontiguousarray(
        np.concatenate([Wq.T, Wq.T], axis=1)).astype(NPBF16)
    wkTd = np.ascontiguousarray(
        np.concatenate([Wk.T, Wk.T], axis=1)).astype(NPBF16)
    wvT = np.ascontiguousarray(Wv.T).astype(NPBF16)
    in_maps = []
    for c in range(8):
        b, h = c // 2, c % 2
        xb = x[b]
        in_maps.append({
            "xT": np.ascontiguousarray(xb.T).astype(NPBF16),
            "xc": np.ascontiguousarray(
                xb.reshape(NKC, KC, D).transpose(1, 0, 2).reshape(KC, S)
            ).astype(NPBF16),
            "xqT": np.ascontiguousarray(
                xb[h * NQ:(h + 1) * NQ].T).astype(NPBF16),
            "wqTd": wqTd, "wkTd": wkTd, "wvT": wvT,
        })
    return in_maps


def run(x, Wq, Wk, Wv, trace=False, **kw):
    global _CACHED_NC
    if _CACHED_NC is None:
        _CACHED_NC = build_nc()
    in_maps = _host_prep(x, Wq, Wk, Wv)
    _log("run_bass_kernel_spmd (includes NEFF compile on first call)")
    res = run_bass_kernel_spmd(
        _CACHED_NC, in_maps, core_ids=list(range(8)), trace=trace, **kw)
    _log("run_bass_kernel_spmd returned")
    full = np.zeros((B, S, D), np.float32)
    for c in range(8):
        b, h = c // 2, c % 2
        full[b, h * NQ:(h + 1) * NQ] = np.asarray(
            res.results[c]["out"]).astype(np.float32).T
    return full, res


def kernel(x, Wq, Wk, Wv):
    full, _ = run(x, Wq, Wk, Wv, trace=False)
    return full
